# revision 22
# baseline (speedup 1.0000x reference)
"""Causal multi-head attention block on 8 trn2 NeuronCores.

Problem: B=2, S=2048, D=768, H=12, Dh=64 (fp32), causal softmax attention
with QKV projections and output projection summed over heads.

Sharding: tensor-parallel over heads x data-parallel over batch.
core c in [0,8): b = c//4, heads = {3g, 3g+1, 3g+2} with g = c%4.
Each core computes the partial output sum over its 3 heads for its batch;
the host sums the 4 partials per batch (the TP all-reduce) and stacks.

Key structure (v3):
  - x staged as 4 per-chunk SBUF tiles (512 columns each, per-partition-
    contiguous DMA) so tile-0 projections depend only on chunk 0; PE
    warm-up matmuls + a dummy activation (exp table preload) run during
    the engine-boot/DMA window.  The Scalar queue carries no input DMAs
    so the exp stream starts as early as possible.
  - Solo-head (h2) attention processed 2 sk-tiles per step: row-packed
    concurrent score matmuls (rows 0:64 / 64:128 via KT3/QT3 low+high
    copies) + ONE exp call per 2 tiles (halves ACT per-call overhead).
  - Softmax normalization split into 4 separately-scheduled stages
    (ones-row copy -> partition broadcast -> reciprocal -> multiply);
    mid-kernel the broadcast rides a DRAM round-trip DMA (latency hidden
    between closure pops), at the tail a K=1 PE outer product.
  - PSUM rings: "s" 2x[128,2,512] scores ping-pong (4 banks), "z" 2
    banks for the head-pair accumulators, "zs" 1 bank shared between the
    solo-z accumulator and early out-proj scratch (allocation-ordered),
    "o" 1 bank for projection evacuations / late out-projs / norm
    broadcasts.  Out-proj work is redistributed into the last pair
    loop's otherwise-empty closure slots.
  - Output stored bf16 (host upcasts + sums partials), one DMA per
    128-row chunk from a [128, 768] stage.
"""

import os
import sys
import types
from collections import deque

import numpy as np
import ml_dtypes

BF16 = ml_dtypes.bfloat16

B, S, D, H, DH = 2, 2048, 768, 12, 64
N_CORES = 8
P = 128
NK = D // P      # 6 contraction chunks
NJ = S // 512    # 4 sq tiles of 512
NI = S // P      # 16 sk tiles of 128
SQT = 512

_PROGRAM = None
LAST_RESULTS = None


def _install_ntff_shim():
    """antenv.axon_hooks is missing in this image; shim it so trace=True works."""
    if "antenv.axon_hooks" in sys.modules:
        return
    try:
        from trn_agent_boot.trn_boot import _ntff_profile_via_ctypes
        m = types.ModuleType("antenv.axon_hooks")
        hook = _ntff_profile_via_ctypes("/opt/axon/libaxon_pjrt.so")
        m.get_axon_ntff_profile_hook = lambda: hook
        m.set_axon_ntff_profile_hook = lambda h: None
        sys.modules["antenv.axon_hooks"] = m
    except Exception:
        pass


def _build_program():
    import concourse.bass as bass
    import concourse.mybir as mybir
    from concourse import bacc
    from concourse.tile import TileContext
    from concourse.bass import ts, ds

    fp32 = mybir.dt.float32
    bf16 = mybir.dt.bfloat16
    Exp = mybir.ActivationFunctionType.Exp
    Mult = mybir.AluOpType.mult

    nc = bacc.Bacc("TRN2", target_bir_lowering=False, debug=False,
                   num_devices=N_CORES)

    xT = nc.dram_tensor("xT", (NJ, P, NK, SQT), bf16, kind="ExternalInput")
    wq2 = nc.dram_tensor("wq2", (P, NK, 128), bf16, kind="ExternalInput")
    wk2 = nc.dram_tensor("wk2", (P, NK, 128), bf16, kind="ExternalInput")
    wqk3 = nc.dram_tensor("wqk3", (P, NK, 128), bf16, kind="ExternalInput")
    wv = nc.dram_tensor("wv", (P, NK, 192), bf16, kind="ExternalInput")
    wo2 = nc.dram_tensor("wo2", (128, D), bf16, kind="ExternalInput")
    wo3 = nc.dram_tensor("wo3", (DH, D), bf16, kind="ExternalInput")
    maskin = nc.dram_tensor("mask", (P, P), bf16, kind="ExternalInput")
    out = nc.dram_tensor("out", (S, D), bf16, kind="ExternalOutput")

    with TileContext(nc) as tc:
        with tc.tile_pool(name="work", bufs=1) as work, \
             tc.tile_pool(name="epool", bufs=6) as epool, \
             tc.tile_pool(name="zsb", bufs=2) as zsb, \
             tc.tile_pool(name="zcol", bufs=3) as zcol, \
             tc.tile_pool(name="dram", bufs=2, space="DRAM") as dram, \
             tc.tile_pool(name="psum", bufs=2, space="PSUM") as psum:

            # ---------------- persistent SBUF tiles ----------------
            QT2 = work.tile([P, S], bf16, name="QT2")   # h0 rows 0:64, h1 64:128
            KT2 = work.tile([P, S], bf16, name="KT2")
            QT3 = work.tile([64, S], bf16, name="QT3")
            QT3hi = work.tile([P, S], bf16, name="QT3hi")  # rows 64:128 = Q3
            KT3hi = work.tile([P, S], bf16, name="KT3hi")  # rows 64:128 = K3
            KT3 = work.tile([64, S], bf16, name="KT3")
            V_all = work.tile([P, NI, 3, 65], bf16, name="V_all")
            xTbs = [work.tile([P, NK, SQT], bf16, name=f"xTb{c}")
                    for c in range(NJ)]
            wq2b = work.tile([P, NK, 128], bf16, name="wq2b")
            wk2b = work.tile([P, NK, 128], bf16, name="wk2b")
            wqk3b = work.tile([P, NK, 128], bf16, name="wqk3b")
            wvb = work.tile([P, NK, 192], bf16, name="wvb")
            wo2b = work.tile([P, D], bf16, name="wo2b")
            wo3b = work.tile([64, D], bf16, name="wo3b")
            maskb = work.tile([P, P], bf16, name="maskb")
            onesb = work.tile([1, 64], bf16, name="onesb")
            warm = work.tile([1, SQT], bf16, name="warm")

            # ---------------- PE warm-up during input DMA ----------------
            nc.vector.memset(warm[:], 1.0)
            nc.vector.memset(onesb[:], 1.0)
            wact = work.tile([1, 16], fp32, name="wact")
            nc.scalar.activation(wact[:], warm[0:1, 0:16], Exp, scale=0.125)
            for w in range(7):
                wp = psum.tile([64, SQT], fp32, tag="o", name="warmp", bufs=1)
                nc.tensor.matmul(wp[:], warm[0:1, 0:64], warm[:],
                                 start=True, stop=True)

            # ---------------- load (already bf16 on host) ----------------
            # x arrives in [P, NK, S] layout; DMA column-chunks of 512,
            # halves split across the sync and scalar queues, so chunk 0
            # (everything tile-0 projections need) lands first.
            # keep the Scalar queue free: its first job must be the exp
            # stream, not x-transfer waits.
            # weights-only on the gpsimd queue (wk2/wq2 first: the first
            # projections need them); all x chunks on the sync HWDGE queue
            # in column order so chunk 0 lands first.
            nc.gpsimd.dma_start(wk2b[:], wk2[:])
            nc.gpsimd.dma_start(wq2b[:], wq2[:])
            nc.sync.dma_start(xTbs[0][:, 0:3, :], xT[0, :, 0:3, :])
            nc.sync.dma_start(xTbs[0][:, 3:6, :], xT[0, :, 3:6, :])
            for c in range(1, NJ):
                nc.sync.dma_start(xTbs[c][:], xT[c, :, :, :])
            nc.gpsimd.dma_start(wqk3b[:], wqk3[:])
            nc.gpsimd.dma_start(wvb[:], wv[:])
            nc.gpsimd.dma_start(wo2b[:], wo2[:])
            nc.gpsimd.dma_start(wo3b[:], wo3[:])
            nc.gpsimd.dma_start(maskb[:], maskin[:])
            nc.vector.memset(V_all[:, :, :, 64], 1.0)

            # ---------------- projection closures ----------------
            # "s"-tag PSUM (2 banks x 2 bufs) is reserved for the scores
            # ping-pong; projection/out-proj closures use the 1-bank "o" tag.
            def proj_q2(t, tag="s", bufs=2):
                p = psum.tile([P, SQT], fp32, tag=tag, name="q2p", bufs=bufs)
                for k in range(NK):
                    nc.tensor.matmul(p[:], wq2b[:, k, :], xTbs[t][:, k, :],
                                     start=(k == 0), stop=(k == NK - 1))
                nc.vector.tensor_copy(QT2[:, ts(t, SQT)], p[:])

            def proj_k2(t, tag="s", bufs=2):
                p = psum.tile([P, SQT], fp32, tag=tag, name="k2p", bufs=bufs)
                for k in range(NK):
                    nc.tensor.matmul(p[:], wk2b[:, k, :], xTbs[t][:, k, :],
                                     start=(k == 0), stop=(k == NK - 1))
                nc.vector.tensor_copy(KT2[:, ts(t, SQT)], p[:])

            def proj_qk3(t, tag="s", bufs=2):
                p = psum.tile([P, SQT], fp32, tag=tag, name="qk3p", bufs=bufs)
                for k in range(NK):
                    nc.tensor.matmul(p[:], wqk3b[:, k, :], xTbs[t][:, k, :],
                                     start=(k == 0), stop=(k == NK - 1))
                nc.vector.tensor_copy(QT3[:, ts(t, SQT)], p[0:64, :])
                nc.vector.tensor_copy(KT3hi[64:128, ts(t, SQT)], p[64:128, :])
                nc.sync.dma_start(KT3[:, ts(t, SQT)],
                                  KT3hi[64:128, ts(t, SQT)])
                nc.sync.dma_start(QT3hi[64:128, ts(t, SQT)],
                                  QT3[:, ts(t, SQT)])

            def proj_v(t, tag="s", bufs=2):
                p = psum.tile([P, 192], fp32, tag=tag, name="vp", bufs=bufs)
                for k in range(NK):
                    nc.tensor.matmul(p[:], xTbs[t // 4][:, k, ts(t % 4, P)],
                                     wvb[:, k, :],
                                     start=(k == 0), stop=(k == NK - 1))
                nc.vector.tensor_copy(V_all[:, t, :, 0:64],
                                      p[:].rearrange("p (h e) -> p h e", h=3))

            # ---------------- upfront: only what scores(j=0,i=0) needs ----
            proj_k2(0)
            proj_q2(0)

            # ---------------- attention ----------------
            # pend_work holds closures popped (traced) inside attn(j)'s
            # loops.  Anything attn(j+1) READS must be traced before attn(j)
            # ends: proj(j+1) is seeded before attn(j) starts; out-proj(j)
            # and norm_solo(j) are appended at the end of attn(j) and
            # consumed during attn(j+1).
            pend_work = deque()

            def pop_work():
                if pend_work:
                    f = pend_work.popleft()
                    if f is not None:
                        f()

            def proj_early(t):
                # alternate the two 1-bank scratch rings ("o"/"zs") so two
                # evacuation chains run in parallel instead of serializing
                # on a single PSUM bank.
                return [lambda: proj_q2(t, "o", 1),
                        lambda: proj_k2(t, "zs", 1),
                        lambda: proj_qk3(t, "o", 1)]

            def proj_late(t):
                # V tiles 4t..4t+3 are first read at pair-j=t's diagonal
                # (pop slot 8t+1), so they can pop early in pair j=t itself.
                return [(lambda v, g: lambda: proj_v(v, g, 1))(
                    v, "zs" if v % 2 else "o")
                    for v in range(4 * t, 4 * t + 4)]

            # j=0 still needs V tiles 0:4 + qk3(0); fold them into the
            # closure stream so attention starts right after q2/k2(0).
            pend_work.extend([(lambda v, g: lambda: proj_v(v, g, 1))(
                v, "zs" if v % 2 else "o") for v in range(4)])
            pend_work.append(lambda: proj_qk3(0, "o", 1))
            pend_work.extend(proj_early(1))

            for j in range(NJ):
                n_i = 4 * j + 4
                n_m = n_i // 2
                sl = ts(j, SQT)
                zp0 = psum.tile([P, SQT], fp32, tag="z", name="zp0", bufs=2)
                zp1 = psum.tile([P, SQT], fp32, tag="z", name="zp1", bufs=2)
                zT2 = zcol.tile([P, SQT], bf16, tag="zT2", name="zT2")
                zT3 = zcol.tile([64, SQT], bf16, tag="zT3", name="zT3")

                def col0_of(i, j=j):
                    return P * (i - 4 * j) if i >= 4 * j else 0

                def s_pair_start(i, j=j):
                    col0 = col0_of(i)
                    s_ps = psum.tile([P, 2, SQT], fp32, tag="s", name="s_ps",
                                     bufs=2)
                    for h in range(2):
                        nc.tensor.matmul(
                            s_ps[:, h, col0:SQT],
                            KT2[64 * h:64 * h + 64, ts(i, P)],
                            QT2[64 * h:64 * h + 64,
                                ds(SQT * j + col0, SQT - col0)],
                            start=True, stop=True)
                    return s_ps, col0

                def s_solo_start(m, j=j):
                    # batch m covers sk tiles i0=2m (rows 0:64 of the PE,
                    # KT3/QT3) and i1=2m+1 (rows 64:128, KT3hi/QT3hi); the
                    # two matmuls run concurrently (distinct row groups).
                    i0, i1 = 2 * m, 2 * m + 1
                    c0, c1 = col0_of(i0), col0_of(i1)
                    s_ps = psum.tile([P, 2, SQT], fp32, tag="s", name="s_ps3",
                                     bufs=2)
                    nc.tensor.matmul(
                        s_ps[:, 0, c0:SQT],
                        KT3[:, ts(i0, P)],
                        QT3[:, ds(SQT * j + c0, SQT - c0)],
                        start=True, stop=True)
                    nc.tensor.matmul(
                        s_ps[:, 1, c1:SQT],
                        KT3hi[64:128, ts(i1, P)],
                        QT3hi[64:128, ds(SQT * j + c1, SQT - c1)],
                        start=True, stop=True)
                    return s_ps, c0, c1

                def exp_mask_pair(i, pend, j=j):
                    s_ps, col0 = pend
                    E_t = epool.tile([P, 2, SQT], bf16, tag="E", name="E_t")
                    nc.scalar.activation(E_t[:, :, col0:SQT],
                                         s_ps[:, :, col0:SQT], Exp, scale=0.125)
                    if i >= 4 * j:
                        nc.vector.tensor_tensor(
                            E_t[:, :, col0:col0 + P], E_t[:, :, col0:col0 + P],
                            maskb[:, None, :].to_broadcast((P, 2, P)), Mult)
                    return E_t, col0

                def exp_mask_solo(m, pend, j=j):
                    s_ps, c0, c1 = pend
                    i0, i1 = 2 * m, 2 * m + 1
                    E_t = epool.tile([P, 2, SQT], bf16, tag="E", name="E_t3")
                    nc.scalar.activation(E_t[:, :, c0:SQT],
                                         s_ps[:, :, c0:SQT], Exp, scale=0.125)
                    if i0 >= 4 * j:
                        nc.vector.tensor_tensor(
                            E_t[:, 0, c0:c0 + P], E_t[:, 0, c0:c0 + P],
                            maskb[:], Mult)
                    if i1 >= 4 * j:
                        nc.vector.tensor_tensor(
                            E_t[:, 1, c1:c1 + P], E_t[:, 1, c1:c1 + P],
                            maskb[:], Mult)
                    return E_t, c0, c1

                # ---- pair i-loop ----
                pend = s_pair_start(0)
                for i in range(n_i):
                    E_t, col0 = exp_mask_pair(i, pend)
                    if i + 1 < n_i:
                        pend = s_pair_start(i + 1)
                    pop_work()
                    for h, zph in ((0, zp0), (1, zp1)):
                        nc.tensor.matmul(
                            zph[0:65, col0:SQT],
                            V_all[:, i, h, :],
                            E_t[:, h, col0:SQT],
                            start=(i == 0), stop=(i == n_i - 1),
                            skip_group_check=True)
                    pop_work()

                # ---- normalize: 1/ones-row straight from PSUM, broadcast
                # across partitions via a K=1 outer-product matmul.
                def norm_stages(z_ps, dst_direct, shift_hi, zT2=zT2,
                                otag="o", obufs=1, via_pe=False):
                    # 4 separately-popped stages so no engine queues an op
                    # whose input is still being produced cross-engine.  The
                    # partition broadcast rides a DRAM round-trip DMA whose
                    # latency hides between pops; at the tail (nothing left
                    # to hide behind) it uses a K=1 PE outer product instead.
                    st = {}

                    def a():
                        st['drow'] = zsb.tile(
                            [1, SQT], bf16 if via_pe else fp32, tag="rb",
                            name="drow")
                        nc.vector.tensor_copy(st['drow'][:], z_ps[64:65, :])
                        if not via_pe:
                            st['dscr'] = dram.tile([1, SQT], fp32,
                                                   name="dscr")
                            nc.gpsimd.dma_start(st['dscr'][:], st['drow'][:])

                    def b():
                        if via_pe:
                            st['dps'] = psum.tile([64, SQT], fp32, tag=otag,
                                                  name="dps", bufs=obufs)
                            nc.tensor.matmul(st['dps'][:], onesb[:],
                                             st['drow'][:], start=True,
                                             stop=True)
                        else:
                            st['draw'] = zsb.tile([64, SQT], fp32, tag="rbw",
                                                  name="draw")
                            nc.gpsimd.dma_start(
                                st['draw'][:],
                                st['dscr'][:].to_broadcast((64, SQT)))

                    def c():
                        if via_pe:
                            st['draw'] = zsb.tile([64, SQT], fp32, tag="rbw",
                                                  name="draw")
                            nc.vector.tensor_copy(st['draw'][:], st['dps'][:])
                        st['rbb'] = zsb.tile([64, SQT], fp32, tag="rbb",
                                             name="rbb")
                        nc.vector.reciprocal_approx_fast(st['rbb'][:],
                                                         st['draw'][:])

                    def d():
                        if shift_hi:
                            z1t = zcol.tile([64, SQT], bf16, tag="z1t",
                                            name="z1t")
                            nc.vector.tensor_tensor(z1t[:], z_ps[0:64, :],
                                                    st['rbb'][:], Mult)
                            nc.sync.dma_start(zT2[64:128, :], z1t[:])
                        else:
                            nc.vector.tensor_tensor(dst_direct, z_ps[0:64, :],
                                                    st['rbb'][:], Mult)

                    return [a, b, c, d]

                n0 = norm_stages(zp0, zT2[0:64, :], False,
                                 via_pe=(j == NJ - 1))
                n1 = norm_stages(zp1, None, True, via_pe=(j == NJ - 1))
                local_work = deque(
                    [n0[0], n1[0], n0[1], n0[2], n1[1], n0[3], n1[2], n1[3]])

                # ---- solo i-loop (2 sk tiles per step) ----
                # zs shares its bank with the out-proj scratch ring: the
                # bank carries oproj(j-1) chains during the pair loop, then
                # zs(j) accumulation here (allocation order serializes it).
                zs = psum.tile([P, SQT], fp32, tag="zs", name="zs", bufs=1)
                pend = s_solo_start(0)
                for m in range(n_m):
                    i0, i1 = 2 * m, 2 * m + 1
                    E_t, c0, c1 = exp_mask_solo(m, pend)
                    if m + 1 < n_m:
                        pend = s_solo_start(m + 1)
                    if local_work:
                        local_work.popleft()()
                    else:
                        pop_work()
                    nc.tensor.matmul(
                        zs[0:65, c0:SQT],
                        V_all[:, i0, 2, :],
                        E_t[:, 0, c0:SQT],
                        start=(m == 0), stop=False,
                        skip_group_check=True)
                    if local_work:
                        local_work.popleft()()
                    else:
                        pop_work()
                    nc.tensor.matmul(
                        zs[0:65, c1:SQT],
                        V_all[:, i1, 2, :],
                        E_t[:, 1, c1:SQT],
                        start=False, stop=(m == n_m - 1),
                        skip_group_check=True)
                    pop_work()

                while local_work:
                    local_work.popleft()()

                # ---- deferred work for the next j ----
                tail = (j == NJ - 1)

                norm_solo_items = norm_stages(
                    zs, zT3[:], False,
                    otag=("s" if tail else "o"),
                    obufs=(2 if tail else 1), via_pe=tail)

                def oproj(c, j=j, zT2=zT2, zT3=zT3, tail=tail):
                    # tail out-projs ping-pong on the freed "s" banks;
                    # oproj(NJ-2) pops during the last solo loop (after
                    # zs(NJ-1) is allocated) so it takes the then-free "o"
                    # ring; otherwise the shared "zs" scratch ring keeps the
                    # "o" ring free for projection evacuations.
                    otag = ("s" if tail else ("o" if j >= NJ - 3 else "zs"))
                    obufs = 2 if tail else 1
                    row = ds(SQT * j + P * c, P)
                    stage = zsb.tile([P, D], bf16, tag="ost", name="ost",
                                     bufs=3)
                    if tail:
                        # one 2-bank tile per chunk: the next chunk's MMs
                        # overlap this chunk's evacuation copies.
                        ot = psum.tile([P, 2, SQT], fp32, tag=otag,
                                       name="otl", bufs=2)
                        o1, o2 = ot[:, 0, :], ot[:, 1, 0:256]
                    else:
                        o1 = psum.tile([P, SQT], fp32, tag=otag, name="o1",
                                       bufs=obufs)[:]
                        o2 = None
                    nc.tensor.matmul(o1, zT2[:, ts(c, P)], wo2b[:, 0:512],
                                     start=True, stop=False,
                                     skip_group_check=True)
                    nc.tensor.matmul(o1, zT3[:, ts(c, P)], wo3b[:, 0:512],
                                     start=False, stop=True,
                                     skip_group_check=True)
                    nc.vector.tensor_copy(stage[:, 0:512], o1)
                    if o2 is None:
                        o2 = psum.tile([P, 256], fp32, tag=otag, name="o2",
                                       bufs=obufs)[:]
                    nc.tensor.matmul(o2, zT2[:, ts(c, P)],
                                     wo2b[:, 512:768],
                                     start=True, stop=False,
                                     skip_group_check=True)
                    nc.tensor.matmul(o2, zT3[:, ts(c, P)],
                                     wo3b[:, 512:768],
                                     start=False, stop=True,
                                     skip_group_check=True)
                    nc.vector.tensor_copy(stage[:, 512:768], o2)
                    (nc.sync if tail else nc.gpsimd).dma_start(
                        out[row, :], stage[:])

                def mk(f, *a):
                    return lambda: f(*a)

                oq = [mk(oproj, 0), mk(oproj, 1), mk(oproj, 2), mk(oproj, 3)]
                if j == NJ - 3:
                    # j2's loops are saturated by proj_early(3): park half
                    # of oproj(1) past all of j2, into pair j3's empty
                    # slots (pops during j2 = 24 pair + ~10 solo = 34).
                    items = list(proj_early(j + 2)) + oq[0:2] + \
                        [None] * 21 + oq[2:4]
                elif j < NJ - 2:
                    rest = iter(proj_early(j + 2))
                    items = [next(rest)]
                    for idx, o_cl in enumerate(oq):
                        items.append(o_cl)
                        nxt = next(rest, None)
                        if nxt is not None:
                            items.append(nxt)
                    items += list(rest)
                elif j == NJ - 2:
                    items = oq
                else:
                    items = oq
                pend_work.extend(norm_solo_items)
                if j + 1 < NJ:
                    pend_work.extend(proj_late(j + 1))
                pend_work.extend(items)

            # ---- drain remaining deferred work (j=3 tail) ----
            while pend_work:
                f = pend_work.popleft()
                if f is not None:
                    f()

    nc.compile()
    return nc


def _get_program():
    global _PROGRAM
    if _PROGRAM is None:
        _PROGRAM = _build_program()
    return _PROGRAM


def kernel(x, W_Q, W_K, W_V, W_O, b_Q, b_K, b_V, b_O):
    global LAST_RESULTS
    _install_ntff_shim()
    from concourse import bass_utils

    x = np.asarray(x, dtype=np.float32)
    W_Q = np.asarray(W_Q, dtype=np.float32)
    W_K = np.asarray(W_K, dtype=np.float32)
    W_V = np.asarray(W_V, dtype=np.float32)
    W_O = np.asarray(W_O, dtype=np.float32)
    b_Q = np.asarray(b_Q, dtype=np.float32)
    b_K = np.asarray(b_K, dtype=np.float32)
    b_V = np.asarray(b_V, dtype=np.float32)
    b_O = np.asarray(b_O, dtype=np.float32)
    assert not (np.any(b_Q) or np.any(b_K) or np.any(b_V)), \
        "kernel assumes zero QKV biases (problem spec fill=zeros)"

    nc = _get_program()

    def bf(a):
        return np.ascontiguousarray(a.astype(BF16))

    def dev_w(w):
        # [768, e] -> [128, 6, e]: chunk k rows on axis 1, partitions on axis 0
        e = w.shape[1]
        return bf(w.reshape(NK, P, e).transpose(1, 0, 2))

    mask = bf(np.triu(np.ones((P, P), dtype=np.float32)))
    # x[b].T is [768, 2048]; stage as [NJ, 128, 6, 512] so each 512-column
    # chunk is contiguous per partition (fast DMA, chunk 0 lands first)
    xTs = [bf(x[b].T.reshape(NK, P, NJ, SQT).transpose(2, 1, 0, 3))
           for b in range(B)]

    in_maps = []
    for c in range(N_CORES):
        b, g = c // 4, c % 4
        hs = [3 * g, 3 * g + 1, 3 * g + 2]
        in_maps.append({
            "xT": xTs[b],
            "wq2": dev_w(np.concatenate([W_Q[hs[0]], W_Q[hs[1]]], axis=1)),
            "wk2": dev_w(np.concatenate([W_K[hs[0]], W_K[hs[1]]], axis=1)),
            "wqk3": dev_w(np.concatenate([W_Q[hs[2]], W_K[hs[2]]], axis=1)),
            "wv": dev_w(np.concatenate(
                [W_V[hs[0]], W_V[hs[1]], W_V[hs[2]]], axis=1)),
            "wo2": bf(np.concatenate([W_O[hs[0]], W_O[hs[1]]], axis=0)),
            "wo3": bf(W_O[hs[2]]),
            "mask": mask,
        })

    res = bass_utils.run_bass_kernel_spmd(
        nc, in_maps, core_ids=list(range(N_CORES)),
        trace=bool(os.environ.get("BASS_TRACE")))
    LAST_RESULTS = res

    parts = [res.results[c]["out"].astype(np.float32) for c in range(N_CORES)]
    full = np.stack([
        parts[0] + parts[1] + parts[2] + parts[3],
        parts[4] + parts[5] + parts[6] + parts[7],
    ], axis=0)
    if np.any(b_O):
        full = full + b_O
    return full.astype(np.float32)


# revision 24
# speedup vs baseline: 1.0211x; 1.0211x over previous
"""Causal multi-head attention block on 8 trn2 NeuronCores.

Problem: B=2, S=2048, D=768, H=12, Dh=64 (fp32), causal softmax attention
with QKV projections and output projection summed over heads.

Sharding: tensor-parallel over heads x data-parallel over batch.
core c in [0,8): b = c//4, heads = {3g, 3g+1, 3g+2} with g = c%4.
Each core computes the partial output sum over its 3 heads for its batch;
the host sums the 4 partials per batch (the TP all-reduce) and stacks.

Key structure (v3):
  - x staged as 4 per-chunk SBUF tiles (512 columns each, per-partition-
    contiguous DMA) so tile-0 projections depend only on chunk 0; PE
    warm-up matmuls + a dummy activation (exp table preload) run during
    the engine-boot/DMA window.  The Scalar queue carries no input DMAs
    so the exp stream starts as early as possible.
  - Solo-head (h2) attention processed 2 sk-tiles per step: row-packed
    concurrent score matmuls (rows 0:64 / 64:128 via KT3/QT3 low+high
    copies) + ONE exp call per 2 tiles (halves ACT per-call overhead).
  - Softmax normalization split into 4 separately-scheduled stages
    (ones-row copy -> partition broadcast -> reciprocal -> multiply);
    mid-kernel the broadcast rides a DRAM round-trip DMA (latency hidden
    between closure pops), at the tail a K=1 PE outer product.
  - PSUM rings: "s" 2x[128,2,512] scores ping-pong (4 banks), "z" 2
    banks for the head-pair accumulators, "zs" 1 bank shared between the
    solo-z accumulator and early out-proj scratch (allocation-ordered),
    "o" 1 bank for projection evacuations / late out-projs / norm
    broadcasts.  Out-proj work is redistributed into the last pair
    loop's otherwise-empty closure slots.
  - Output stored bf16 (host upcasts + sums partials), one DMA per
    128-row chunk from a [128, 768] stage.
"""

import os
import sys
import types
from collections import deque

import numpy as np
import ml_dtypes

BF16 = ml_dtypes.bfloat16

B, S, D, H, DH = 2, 2048, 768, 12, 64
N_CORES = 8
P = 128
NK = D // P      # 6 contraction chunks
NJ = S // 512    # 4 sq tiles of 512
NI = S // P      # 16 sk tiles of 128
SQT = 512

_PROGRAM = None
LAST_RESULTS = None


def _install_ntff_shim():
    """antenv.axon_hooks is missing in this image; shim it so trace=True works."""
    if "antenv.axon_hooks" in sys.modules:
        return
    try:
        from trn_agent_boot.trn_boot import _ntff_profile_via_ctypes
        m = types.ModuleType("antenv.axon_hooks")
        hook = _ntff_profile_via_ctypes("/opt/axon/libaxon_pjrt.so")
        m.get_axon_ntff_profile_hook = lambda: hook
        m.set_axon_ntff_profile_hook = lambda h: None
        sys.modules["antenv.axon_hooks"] = m
    except Exception:
        pass


def _build_program():
    import concourse.bass as bass
    import concourse.mybir as mybir
    from concourse import bacc
    from concourse.tile import TileContext
    from concourse.bass import ts, ds

    fp32 = mybir.dt.float32
    bf16 = mybir.dt.bfloat16
    Exp = mybir.ActivationFunctionType.Exp
    Mult = mybir.AluOpType.mult

    nc = bacc.Bacc("TRN2", target_bir_lowering=False, debug=False,
                   num_devices=N_CORES)

    xT = nc.dram_tensor("xT", (NJ, P, NK, SQT), bf16, kind="ExternalInput")
    wq2 = nc.dram_tensor("wq2", (P, NK, 128), bf16, kind="ExternalInput")
    wk2 = nc.dram_tensor("wk2", (P, NK, 128), bf16, kind="ExternalInput")
    wqk3 = nc.dram_tensor("wqk3", (P, NK, 128), bf16, kind="ExternalInput")
    wv = nc.dram_tensor("wv", (P, NK, 192), bf16, kind="ExternalInput")
    wo2 = nc.dram_tensor("wo2", (128, D), bf16, kind="ExternalInput")
    wo3 = nc.dram_tensor("wo3", (DH, D), bf16, kind="ExternalInput")
    maskin = nc.dram_tensor("mask", (P, P), bf16, kind="ExternalInput")
    out = nc.dram_tensor("out", (S, D), bf16, kind="ExternalOutput")

    with TileContext(nc) as tc:
        with tc.tile_pool(name="work", bufs=1) as work, \
             tc.tile_pool(name="epool", bufs=6) as epool, \
             tc.tile_pool(name="zsb", bufs=2) as zsb, \
             tc.tile_pool(name="zcol", bufs=3) as zcol, \
             tc.tile_pool(name="dram", bufs=2, space="DRAM") as dram, \
             tc.tile_pool(name="psum", bufs=2, space="PSUM") as psum:

            # ---------------- persistent SBUF tiles ----------------
            QT2 = work.tile([P, S], bf16, name="QT2")   # h0 rows 0:64, h1 64:128
            KT2 = work.tile([P, S], bf16, name="KT2")
            QT3 = work.tile([64, S], bf16, name="QT3")
            QT3hi = work.tile([P, S], bf16, name="QT3hi")  # rows 64:128 = Q3
            KT3hi = work.tile([P, S], bf16, name="KT3hi")  # rows 64:128 = K3
            KT3 = work.tile([64, S], bf16, name="KT3")
            V_all = work.tile([P, NI, 3, 65], bf16, name="V_all")
            xTbs = [work.tile([P, NK, SQT], bf16, name=f"xTb{c}")
                    for c in range(NJ)]
            wq2b = work.tile([P, NK, 128], bf16, name="wq2b")
            wk2b = work.tile([P, NK, 128], bf16, name="wk2b")
            wqk3b = work.tile([P, NK, 128], bf16, name="wqk3b")
            wvb = work.tile([P, NK, 192], bf16, name="wvb")
            wo2b = work.tile([P, D], bf16, name="wo2b")
            wo3b = work.tile([64, D], bf16, name="wo3b")
            maskb = work.tile([P, P], bf16, name="maskb")
            onesb = work.tile([1, 64], bf16, name="onesb")
            warm = work.tile([1, SQT], bf16, name="warm")

            # ---------------- PE warm-up during input DMA ----------------
            nc.vector.memset(warm[:], 1.0)
            nc.vector.memset(onesb[:], 1.0)
            wact = work.tile([1, 16], fp32, name="wact")
            nc.scalar.activation(wact[:], warm[0:1, 0:16], Exp, scale=0.125)
            for w in range(7):
                wp = psum.tile([64, SQT], fp32, tag="o", name="warmp", bufs=1)
                nc.tensor.matmul(wp[:], warm[0:1, 0:64], warm[:],
                                 start=True, stop=True)

            # ---------------- load (already bf16 on host) ----------------
            # x arrives in [P, NK, S] layout; DMA column-chunks of 512,
            # halves split across the sync and scalar queues, so chunk 0
            # (everything tile-0 projections need) lands first.
            # keep the Scalar queue free: its first job must be the exp
            # stream, not x-transfer waits.
            # weights-only on the gpsimd queue (wk2/wq2 first: the first
            # projections need them); all x chunks on the sync HWDGE queue
            # in column order so chunk 0 lands first.
            nc.gpsimd.dma_start(wk2b[:], wk2[:])
            nc.gpsimd.dma_start(wq2b[:], wq2[:])
            nc.sync.dma_start(xTbs[0][:, 0:3, :], xT[0, :, 0:3, :])
            nc.sync.dma_start(xTbs[0][:, 3:6, :], xT[0, :, 3:6, :])
            for c in range(1, NJ):
                nc.sync.dma_start(xTbs[c][:], xT[c, :, :, :])
            nc.gpsimd.dma_start(wqk3b[:], wqk3[:])
            nc.gpsimd.dma_start(wvb[:], wv[:])
            nc.gpsimd.dma_start(wo2b[:], wo2[:])
            nc.gpsimd.dma_start(wo3b[:], wo3[:])
            nc.gpsimd.dma_start(maskb[:], maskin[:])
            nc.vector.memset(V_all[:, :, :, 64], 1.0)

            # ---------------- projection closures ----------------
            # "s"-tag PSUM (2 banks x 2 bufs) is reserved for the scores
            # ping-pong; projection/out-proj closures use the 1-bank "o" tag.
            def proj_q2(t, tag="s", bufs=2):
                p = psum.tile([P, SQT], fp32, tag=tag, name="q2p", bufs=bufs)
                for k in range(NK):
                    nc.tensor.matmul(p[:], wq2b[:, k, :], xTbs[t][:, k, :],
                                     start=(k == 0), stop=(k == NK - 1))
                nc.vector.tensor_copy(QT2[:, ts(t, SQT)], p[:])

            def proj_k2(t, tag="s", bufs=2):
                p = psum.tile([P, SQT], fp32, tag=tag, name="k2p", bufs=bufs)
                for k in range(NK):
                    nc.tensor.matmul(p[:], wk2b[:, k, :], xTbs[t][:, k, :],
                                     start=(k == 0), stop=(k == NK - 1))
                nc.vector.tensor_copy(KT2[:, ts(t, SQT)], p[:])

            def proj_qk3(t, tag="s", bufs=2):
                p = psum.tile([P, SQT], fp32, tag=tag, name="qk3p", bufs=bufs)
                for k in range(NK):
                    nc.tensor.matmul(p[:], wqk3b[:, k, :], xTbs[t][:, k, :],
                                     start=(k == 0), stop=(k == NK - 1))
                nc.vector.tensor_copy(QT3[:, ts(t, SQT)], p[0:64, :])
                nc.vector.tensor_copy(KT3hi[64:128, ts(t, SQT)], p[64:128, :])
                nc.sync.dma_start(KT3[:, ts(t, SQT)],
                                  KT3hi[64:128, ts(t, SQT)])
                nc.sync.dma_start(QT3hi[64:128, ts(t, SQT)],
                                  QT3[:, ts(t, SQT)])

            def proj_v(t, tag="s", bufs=2):
                p = psum.tile([P, 192], fp32, tag=tag, name="vp", bufs=bufs)
                for k in range(NK):
                    nc.tensor.matmul(p[:], xTbs[t // 4][:, k, ts(t % 4, P)],
                                     wvb[:, k, :],
                                     start=(k == 0), stop=(k == NK - 1))
                nc.vector.tensor_copy(V_all[:, t, :, 0:64],
                                      p[:].rearrange("p (h e) -> p h e", h=3))

            # ---------------- upfront: only what scores(j=0,i=0) needs ----
            proj_k2(0)
            proj_q2(0)

            # ---------------- attention ----------------
            # pend_work holds closures popped (traced) inside attn(j)'s
            # loops.  Anything attn(j+1) READS must be traced before attn(j)
            # ends: proj(j+1) is seeded before attn(j) starts; out-proj(j)
            # and norm_solo(j) are appended at the end of attn(j) and
            # consumed during attn(j+1).
            pend_work = deque()

            def pop_work():
                if pend_work:
                    f = pend_work.popleft()
                    if f is not None:
                        f()

            def proj_items(t):
                # alternate the two 1-bank scratch rings ("o"/"zs") so two
                # evacuation chains run in parallel instead of serializing
                # on a single PSUM bank.
                return [lambda: proj_q2(t, "o", 1),
                        lambda: proj_k2(t, "zs", 1),
                        lambda: proj_qk3(t, "o", 1)] + \
                       [(lambda v, g: lambda: proj_v(v, g, 1))(
                           v, "zs" if v % 2 else "o")
                        for v in range(4 * t, 4 * t + 4)]

            # j=0 still needs V tiles 0:4 + qk3(0); fold them into the
            # closure stream so attention starts right after q2/k2(0).
            pend_work.extend([(lambda v, g: lambda: proj_v(v, g, 1))(
                v, "zs" if v % 2 else "o") for v in range(4)])
            pend_work.append(lambda: proj_qk3(0, "o", 1))
            pend_work.extend(proj_items(1))

            for j in range(NJ):
                n_i = 4 * j + 4
                n_m = n_i // 2
                sl = ts(j, SQT)
                zp0 = psum.tile([P, SQT], fp32, tag="z", name="zp0", bufs=2)
                zp1 = psum.tile([P, SQT], fp32, tag="z", name="zp1", bufs=2)
                zT2 = zcol.tile([P, SQT], bf16, tag="zT2", name="zT2")
                zT3 = zcol.tile([64, SQT], bf16, tag="zT3", name="zT3")

                def col0_of(i, j=j):
                    return P * (i - 4 * j) if i >= 4 * j else 0

                def s_pair_start(i, j=j):
                    col0 = col0_of(i)
                    s_ps = psum.tile([P, 2, SQT], fp32, tag="s", name="s_ps",
                                     bufs=2)
                    for h in range(2):
                        nc.tensor.matmul(
                            s_ps[:, h, col0:SQT],
                            KT2[64 * h:64 * h + 64, ts(i, P)],
                            QT2[64 * h:64 * h + 64,
                                ds(SQT * j + col0, SQT - col0)],
                            start=True, stop=True)
                    return s_ps, col0

                def s_solo_start(m, j=j):
                    # batch m covers sk tiles i0=2m (rows 0:64 of the PE,
                    # KT3/QT3) and i1=2m+1 (rows 64:128, KT3hi/QT3hi); the
                    # two matmuls run concurrently (distinct row groups).
                    i0, i1 = 2 * m, 2 * m + 1
                    c0, c1 = col0_of(i0), col0_of(i1)
                    s_ps = psum.tile([P, 2, SQT], fp32, tag="s", name="s_ps3",
                                     bufs=2)
                    nc.tensor.matmul(
                        s_ps[:, 0, c0:SQT],
                        KT3[:, ts(i0, P)],
                        QT3[:, ds(SQT * j + c0, SQT - c0)],
                        start=True, stop=True)
                    nc.tensor.matmul(
                        s_ps[:, 1, c1:SQT],
                        KT3hi[64:128, ts(i1, P)],
                        QT3hi[64:128, ds(SQT * j + c1, SQT - c1)],
                        start=True, stop=True)
                    return s_ps, c0, c1

                def exp_mask_pair(i, pend, j=j):
                    s_ps, col0 = pend
                    E_t = epool.tile([P, 2, SQT], bf16, tag="E", name="E_t")
                    nc.scalar.activation(E_t[:, :, col0:SQT],
                                         s_ps[:, :, col0:SQT], Exp, scale=0.125)
                    if i >= 4 * j:
                        nc.vector.tensor_tensor(
                            E_t[:, :, col0:col0 + P], E_t[:, :, col0:col0 + P],
                            maskb[:, None, :].to_broadcast((P, 2, P)), Mult)
                    return E_t, col0

                def exp_mask_solo(m, pend, j=j):
                    s_ps, c0, c1 = pend
                    i0, i1 = 2 * m, 2 * m + 1
                    E_t = epool.tile([P, 2, SQT], bf16, tag="E", name="E_t3")
                    nc.scalar.activation(E_t[:, :, c0:SQT],
                                         s_ps[:, :, c0:SQT], Exp, scale=0.125)
                    if i0 >= 4 * j:
                        nc.vector.tensor_tensor(
                            E_t[:, 0, c0:c0 + P], E_t[:, 0, c0:c0 + P],
                            maskb[:], Mult)
                    if i1 >= 4 * j:
                        nc.vector.tensor_tensor(
                            E_t[:, 1, c1:c1 + P], E_t[:, 1, c1:c1 + P],
                            maskb[:], Mult)
                    return E_t, c0, c1

                # ---- pair i-loop ----
                pend = s_pair_start(0)
                for i in range(n_i):
                    E_t, col0 = exp_mask_pair(i, pend)
                    if i + 1 < n_i:
                        pend = s_pair_start(i + 1)
                    pop_work()
                    for h, zph in ((0, zp0), (1, zp1)):
                        nc.tensor.matmul(
                            zph[0:65, col0:SQT],
                            V_all[:, i, h, :],
                            E_t[:, h, col0:SQT],
                            start=(i == 0), stop=(i == n_i - 1),
                            skip_group_check=True)
                    pop_work()

                # ---- normalize: 1/ones-row straight from PSUM, broadcast
                # across partitions via a K=1 outer-product matmul.
                def norm_stages(z_ps, dst_direct, shift_hi, zT2=zT2,
                                otag="o", obufs=1, via_pe=False):
                    # 4 separately-popped stages so no engine queues an op
                    # whose input is still being produced cross-engine.  The
                    # partition broadcast rides a DRAM round-trip DMA whose
                    # latency hides between pops; at the tail (nothing left
                    # to hide behind) it uses a K=1 PE outer product instead.
                    st = {}

                    def a():
                        st['drow'] = zsb.tile(
                            [1, SQT], bf16 if via_pe else fp32, tag="rb",
                            name="drow")
                        nc.vector.tensor_copy(st['drow'][:], z_ps[64:65, :])
                        if not via_pe:
                            st['dscr'] = dram.tile([1, SQT], fp32,
                                                   name="dscr")
                            nc.gpsimd.dma_start(st['dscr'][:], st['drow'][:])

                    def b():
                        if via_pe:
                            st['dps'] = psum.tile([64, SQT], fp32, tag=otag,
                                                  name="dps", bufs=obufs)
                            nc.tensor.matmul(st['dps'][:], onesb[:],
                                             st['drow'][:], start=True,
                                             stop=True)
                        else:
                            st['draw'] = zsb.tile([64, SQT], fp32, tag="rbw",
                                                  name="draw")
                            nc.gpsimd.dma_start(
                                st['draw'][:],
                                st['dscr'][:].to_broadcast((64, SQT)))

                    def c():
                        if via_pe:
                            st['draw'] = zsb.tile([64, SQT], fp32, tag="rbw",
                                                  name="draw")
                            nc.vector.tensor_copy(st['draw'][:], st['dps'][:])
                        st['rbb'] = zsb.tile([64, SQT], fp32, tag="rbb",
                                             name="rbb")
                        nc.vector.reciprocal_approx_fast(st['rbb'][:],
                                                         st['draw'][:])

                    def d():
                        if shift_hi:
                            z1t = zcol.tile([64, SQT], bf16, tag="z1t",
                                            name="z1t")
                            nc.vector.tensor_tensor(z1t[:], z_ps[0:64, :],
                                                    st['rbb'][:], Mult)
                            nc.sync.dma_start(zT2[64:128, :], z1t[:])
                        else:
                            nc.vector.tensor_tensor(dst_direct, z_ps[0:64, :],
                                                    st['rbb'][:], Mult)

                    return [a, b, c, d]

                n0 = norm_stages(zp0, zT2[0:64, :], False,
                                 via_pe=(j == NJ - 1))
                n1 = norm_stages(zp1, None, True, via_pe=(j == NJ - 1))
                local_work = deque(
                    [n0[0], n1[0], n0[1], n0[2], n1[1], n0[3], n1[2], n1[3]])

                # ---- solo i-loop (2 sk tiles per step) ----
                # zs shares its bank with the out-proj scratch ring: the
                # bank carries oproj(j-1) chains during the pair loop, then
                # zs(j) accumulation here (allocation order serializes it).
                zs = psum.tile([P, SQT], fp32, tag="zs", name="zs", bufs=1)
                pend = s_solo_start(0)
                for m in range(n_m):
                    i0, i1 = 2 * m, 2 * m + 1
                    E_t, c0, c1 = exp_mask_solo(m, pend)
                    if m + 1 < n_m:
                        pend = s_solo_start(m + 1)
                    if local_work:
                        local_work.popleft()()
                    else:
                        pop_work()
                    nc.tensor.matmul(
                        zs[0:65, c0:SQT],
                        V_all[:, i0, 2, :],
                        E_t[:, 0, c0:SQT],
                        start=(m == 0), stop=False,
                        skip_group_check=True)
                    if local_work:
                        local_work.popleft()()
                    else:
                        pop_work()
                    nc.tensor.matmul(
                        zs[0:65, c1:SQT],
                        V_all[:, i1, 2, :],
                        E_t[:, 1, c1:SQT],
                        start=False, stop=(m == n_m - 1),
                        skip_group_check=True)
                    pop_work()

                while local_work:
                    local_work.popleft()()

                # ---- deferred work for the next j ----
                tail = (j == NJ - 1)

                norm_solo_items = norm_stages(
                    zs, zT3[:], False,
                    otag=("s" if tail else "o"),
                    obufs=(2 if tail else 1), via_pe=tail)

                def oproj(c, j=j, zT2=zT2, zT3=zT3, tail=tail):
                    # tail out-projs ping-pong on the freed "s" banks;
                    # oproj(NJ-2) pops during the last solo loop (after
                    # zs(NJ-1) is allocated) so it takes the then-free "o"
                    # ring; otherwise the shared "zs" scratch ring keeps the
                    # "o" ring free for projection evacuations.
                    otag = ("s" if tail else ("o" if j >= NJ - 3 else "zs"))
                    obufs = 2 if tail else 1
                    row = ds(SQT * j + P * c, P)
                    stage = zsb.tile([P, D], bf16, tag="ost", name="ost",
                                     bufs=3)
                    if tail:
                        # one 2-bank tile per chunk: the next chunk's MMs
                        # overlap this chunk's evacuation copies.
                        ot = psum.tile([P, 2, SQT], fp32, tag=otag,
                                       name="otl", bufs=2)
                        o1, o2 = ot[:, 0, :], ot[:, 1, 0:256]
                    else:
                        o1 = psum.tile([P, SQT], fp32, tag=otag, name="o1",
                                       bufs=obufs)[:]
                        o2 = None
                    nc.tensor.matmul(o1, zT2[:, ts(c, P)], wo2b[:, 0:512],
                                     start=True, stop=False,
                                     skip_group_check=True)
                    nc.tensor.matmul(o1, zT3[:, ts(c, P)], wo3b[:, 0:512],
                                     start=False, stop=True,
                                     skip_group_check=True)
                    nc.vector.tensor_copy(stage[:, 0:512], o1)
                    if o2 is None:
                        o2 = psum.tile([P, 256], fp32, tag=otag, name="o2",
                                       bufs=obufs)[:]
                    nc.tensor.matmul(o2, zT2[:, ts(c, P)],
                                     wo2b[:, 512:768],
                                     start=True, stop=False,
                                     skip_group_check=True)
                    nc.tensor.matmul(o2, zT3[:, ts(c, P)],
                                     wo3b[:, 512:768],
                                     start=False, stop=True,
                                     skip_group_check=True)
                    nc.vector.tensor_copy(stage[:, 512:768], o2)
                    (nc.sync if tail else nc.gpsimd).dma_start(
                        out[row, :], stage[:])

                def mk(f, *a):
                    return lambda: f(*a)

                oq = [mk(oproj, 0), mk(oproj, 1), mk(oproj, 2), mk(oproj, 3)]
                if j == NJ - 3:
                    # j2's loops are already closure-saturated by proj(3):
                    # park half of oproj(1) past them, into pair j3's many
                    # empty slots (pops during j2 = 24 pair + ~10 solo).
                    items = list(proj_items(j + 2)) + oq[0:2] +                         [None] * 12 + oq[2:4]
                elif j < NJ - 2:
                    rest = iter(proj_items(j + 2))
                    items = [next(rest), next(rest)]
                    for idx, o_cl in enumerate(oq):
                        items.append(o_cl)
                        nxt = next(rest, None)
                        if nxt is not None:
                            items.append(nxt)
                    items += list(rest)
                elif j == NJ - 2:
                    items = [None] * 2 + oq
                else:
                    items = oq
                pend_work.extend(norm_solo_items)
                pend_work.extend(items)

            # ---- drain remaining deferred work (j=3 tail) ----
            while pend_work:
                f = pend_work.popleft()
                if f is not None:
                    f()

    nc.compile()
    return nc


def _get_program():
    global _PROGRAM
    if _PROGRAM is None:
        _PROGRAM = _build_program()
    return _PROGRAM


def kernel(x, W_Q, W_K, W_V, W_O, b_Q, b_K, b_V, b_O):
    global LAST_RESULTS
    _install_ntff_shim()
    from concourse import bass_utils

    x = np.asarray(x, dtype=np.float32)
    W_Q = np.asarray(W_Q, dtype=np.float32)
    W_K = np.asarray(W_K, dtype=np.float32)
    W_V = np.asarray(W_V, dtype=np.float32)
    W_O = np.asarray(W_O, dtype=np.float32)
    b_Q = np.asarray(b_Q, dtype=np.float32)
    b_K = np.asarray(b_K, dtype=np.float32)
    b_V = np.asarray(b_V, dtype=np.float32)
    b_O = np.asarray(b_O, dtype=np.float32)
    assert not (np.any(b_Q) or np.any(b_K) or np.any(b_V)), \
        "kernel assumes zero QKV biases (problem spec fill=zeros)"

    nc = _get_program()

    def bf(a):
        return np.ascontiguousarray(a.astype(BF16))

    def dev_w(w):
        # [768, e] -> [128, 6, e]: chunk k rows on axis 1, partitions on axis 0
        e = w.shape[1]
        return bf(w.reshape(NK, P, e).transpose(1, 0, 2))

    mask = bf(np.triu(np.ones((P, P), dtype=np.float32)))
    # x[b].T is [768, 2048]; stage as [NJ, 128, 6, 512] so each 512-column
    # chunk is contiguous per partition (fast DMA, chunk 0 lands first)
    xTs = [bf(x[b].T.reshape(NK, P, NJ, SQT).transpose(2, 1, 0, 3))
           for b in range(B)]

    in_maps = []
    for c in range(N_CORES):
        b, g = c // 4, c % 4
        hs = [3 * g, 3 * g + 1, 3 * g + 2]
        in_maps.append({
            "xT": xTs[b],
            "wq2": dev_w(np.concatenate([W_Q[hs[0]], W_Q[hs[1]]], axis=1)),
            "wk2": dev_w(np.concatenate([W_K[hs[0]], W_K[hs[1]]], axis=1)),
            "wqk3": dev_w(np.concatenate([W_Q[hs[2]], W_K[hs[2]]], axis=1)),
            "wv": dev_w(np.concatenate(
                [W_V[hs[0]], W_V[hs[1]], W_V[hs[2]]], axis=1)),
            "wo2": bf(np.concatenate([W_O[hs[0]], W_O[hs[1]]], axis=0)),
            "wo3": bf(W_O[hs[2]]),
            "mask": mask,
        })

    res = bass_utils.run_bass_kernel_spmd(
        nc, in_maps, core_ids=list(range(N_CORES)),
        trace=bool(os.environ.get("BASS_TRACE")))
    LAST_RESULTS = res

    parts = [res.results[c]["out"].astype(np.float32) for c in range(N_CORES)]
    full = np.stack([
        parts[0] + parts[1] + parts[2] + parts[3],
        parts[4] + parts[5] + parts[6] + parts[7],
    ], axis=0)
    if np.any(b_O):
        full = full + b_O
    return full.astype(np.float32)


# revision 25
# speedup vs baseline: 1.0516x; 1.0299x over previous
"""Causal multi-head attention block on 8 trn2 NeuronCores.

Problem: B=2, S=2048, D=768, H=12, Dh=64 (fp32), causal softmax attention
with QKV projections and output projection summed over heads.

Sharding: tensor-parallel over heads x data-parallel over batch.
core c in [0,8): b = c//4, heads = {3g, 3g+1, 3g+2} with g = c%4.
Each core computes the partial output sum over its 3 heads for its batch;
the host sums the 4 partials per batch (the TP all-reduce) and stacks.

Key structure (v3):
  - x staged as 4 per-chunk SBUF tiles (512 columns each, per-partition-
    contiguous DMA) so tile-0 projections depend only on chunk 0; PE
    warm-up matmuls + a dummy activation (exp table preload) run during
    the engine-boot/DMA window.  The Scalar queue carries no input DMAs
    so the exp stream starts as early as possible.
  - Solo-head (h2) attention processed 2 sk-tiles per step: row-packed
    concurrent score matmuls (rows 0:64 / 64:128 via KT3/QT3 low+high
    copies) + ONE exp call per 2 tiles (halves ACT per-call overhead).
  - Softmax normalization split into 4 separately-scheduled stages
    (ones-row copy -> partition broadcast -> reciprocal -> multiply);
    mid-kernel the broadcast rides a DRAM round-trip DMA (latency hidden
    between closure pops), at the tail a K=1 PE outer product.
  - PSUM rings: "s" 2x[128,2,512] scores ping-pong (4 banks), "z" 2
    banks for the head-pair accumulators, "zs" 1 bank shared between the
    solo-z accumulator and early out-proj scratch (allocation-ordered),
    "o" 1 bank for projection evacuations / late out-projs / norm
    broadcasts.  Out-proj work is redistributed into the last pair
    loop's otherwise-empty closure slots.
  - Output stored bf16 (host upcasts + sums partials), one DMA per
    128-row chunk from a [128, 768] stage.
"""

import os
import sys
import types
from collections import deque

import numpy as np
import ml_dtypes

BF16 = ml_dtypes.bfloat16

B, S, D, H, DH = 2, 2048, 768, 12, 64
N_CORES = 8
P = 128
NK = D // P      # 6 contraction chunks
NJ = S // 512    # 4 sq tiles of 512
NI = S // P      # 16 sk tiles of 128
SQT = 512

_PROGRAM = None
LAST_RESULTS = None


def _install_ntff_shim():
    """antenv.axon_hooks is missing in this image; shim it so trace=True works."""
    if "antenv.axon_hooks" in sys.modules:
        return
    try:
        from trn_agent_boot.trn_boot import _ntff_profile_via_ctypes
        m = types.ModuleType("antenv.axon_hooks")
        hook = _ntff_profile_via_ctypes("/opt/axon/libaxon_pjrt.so")
        m.get_axon_ntff_profile_hook = lambda: hook
        m.set_axon_ntff_profile_hook = lambda h: None
        sys.modules["antenv.axon_hooks"] = m
    except Exception:
        pass


def _build_program():
    import concourse.bass as bass
    import concourse.mybir as mybir
    from concourse import bacc
    from concourse.tile import TileContext
    from concourse.bass import ts, ds

    fp32 = mybir.dt.float32
    bf16 = mybir.dt.bfloat16
    Exp = mybir.ActivationFunctionType.Exp
    Mult = mybir.AluOpType.mult

    nc = bacc.Bacc("TRN2", target_bir_lowering=False, debug=False,
                   num_devices=N_CORES)

    xT = nc.dram_tensor("xT", (NJ, P, NK, SQT), bf16, kind="ExternalInput")
    wq2 = nc.dram_tensor("wq2", (P, NK, 128), bf16, kind="ExternalInput")
    wk2 = nc.dram_tensor("wk2", (P, NK, 128), bf16, kind="ExternalInput")
    wqk3 = nc.dram_tensor("wqk3", (P, NK, 128), bf16, kind="ExternalInput")
    wv = nc.dram_tensor("wv", (P, NK, 192), bf16, kind="ExternalInput")
    wo2 = nc.dram_tensor("wo2", (128, D), bf16, kind="ExternalInput")
    wo3 = nc.dram_tensor("wo3", (DH, D), bf16, kind="ExternalInput")
    maskin = nc.dram_tensor("mask", (P, P), bf16, kind="ExternalInput")
    out = nc.dram_tensor("out", (S, D), bf16, kind="ExternalOutput")

    with TileContext(nc) as tc:
        with tc.tile_pool(name="work", bufs=1) as work, \
             tc.tile_pool(name="epool", bufs=8) as epool, \
             tc.tile_pool(name="zsb", bufs=3) as zsb, \
             tc.tile_pool(name="zcol", bufs=3) as zcol, \
             tc.tile_pool(name="dram", bufs=2, space="DRAM") as dram, \
             tc.tile_pool(name="psum", bufs=2, space="PSUM") as psum:

            # ---------------- persistent SBUF tiles ----------------
            QT2 = work.tile([P, S], bf16, name="QT2")   # h0 rows 0:64, h1 64:128
            KT2 = work.tile([P, S], bf16, name="KT2")
            QT3 = work.tile([64, S], bf16, name="QT3")
            QT3hi = work.tile([P, S], bf16, name="QT3hi")  # rows 64:128 = Q3
            KT3hi = work.tile([P, S], bf16, name="KT3hi")  # rows 64:128 = K3
            KT3 = work.tile([64, S], bf16, name="KT3")
            V_all = work.tile([P, NI, 3, 65], bf16, name="V_all")
            xTbs = [work.tile([P, NK, SQT], bf16, name=f"xTb{c}")
                    for c in range(NJ)]
            wq2b = work.tile([P, NK, 128], bf16, name="wq2b")
            wk2b = work.tile([P, NK, 128], bf16, name="wk2b")
            wqk3b = work.tile([P, NK, 128], bf16, name="wqk3b")
            wvb = work.tile([P, NK, 192], bf16, name="wvb")
            wo2b = work.tile([P, D], bf16, name="wo2b")
            wo3b = work.tile([64, D], bf16, name="wo3b")
            maskb = work.tile([P, P], bf16, name="maskb")
            onesb = work.tile([1, 64], bf16, name="onesb")
            warm = work.tile([1, SQT], bf16, name="warm")

            # ---------------- PE warm-up during input DMA ----------------
            nc.vector.memset(warm[:], 1.0)
            nc.vector.memset(onesb[:], 1.0)
            wact = work.tile([1, 16], fp32, name="wact")
            nc.scalar.activation(wact[:], warm[0:1, 0:16], Exp, scale=0.125)
            for w in range(7):
                wp = psum.tile([64, SQT], fp32, tag="o", name="warmp", bufs=1)
                nc.tensor.matmul(wp[:], warm[0:1, 0:64], warm[:],
                                 start=True, stop=True)

            # ---------------- load (already bf16 on host) ----------------
            # x arrives in [P, NK, S] layout; DMA column-chunks of 512,
            # halves split across the sync and scalar queues, so chunk 0
            # (everything tile-0 projections need) lands first.
            # keep the Scalar queue free: its first job must be the exp
            # stream, not x-transfer waits.
            # weights-only on the gpsimd queue (wk2/wq2 first: the first
            # projections need them); all x chunks on the sync HWDGE queue
            # in column order so chunk 0 lands first.
            nc.gpsimd.dma_start(wk2b[:], wk2[:])
            nc.gpsimd.dma_start(wq2b[:], wq2[:])
            nc.sync.dma_start(xTbs[0][:, 0:3, :], xT[0, :, 0:3, :])
            nc.sync.dma_start(xTbs[0][:, 3:6, :], xT[0, :, 3:6, :])
            for c in range(1, NJ):
                nc.sync.dma_start(xTbs[c][:], xT[c, :, :, :])
            nc.gpsimd.dma_start(wqk3b[:], wqk3[:])
            nc.gpsimd.dma_start(wvb[:], wv[:])
            nc.gpsimd.dma_start(wo2b[:], wo2[:])
            nc.gpsimd.dma_start(wo3b[:], wo3[:])
            nc.gpsimd.dma_start(maskb[:], maskin[:])
            nc.vector.memset(V_all[:, :, :, 64], 1.0)

            # ---------------- projection closures ----------------
            # "s"-tag PSUM (2 banks x 2 bufs) is reserved for the scores
            # ping-pong; projection/out-proj closures use the 1-bank "o" tag.
            def proj_q2(t, tag="s", bufs=2):
                p = psum.tile([P, SQT], fp32, tag=tag, name="q2p", bufs=bufs)
                for k in range(NK):
                    nc.tensor.matmul(p[:], wq2b[:, k, :], xTbs[t][:, k, :],
                                     start=(k == 0), stop=(k == NK - 1))
                nc.vector.tensor_copy(QT2[:, ts(t, SQT)], p[:])

            def proj_k2(t, tag="s", bufs=2):
                p = psum.tile([P, SQT], fp32, tag=tag, name="k2p", bufs=bufs)
                for k in range(NK):
                    nc.tensor.matmul(p[:], wk2b[:, k, :], xTbs[t][:, k, :],
                                     start=(k == 0), stop=(k == NK - 1))
                nc.vector.tensor_copy(KT2[:, ts(t, SQT)], p[:])

            def proj_qk3(t, tag="s", bufs=2):
                p = psum.tile([P, SQT], fp32, tag=tag, name="qk3p", bufs=bufs)
                for k in range(NK):
                    nc.tensor.matmul(p[:], wqk3b[:, k, :], xTbs[t][:, k, :],
                                     start=(k == 0), stop=(k == NK - 1))
                nc.vector.tensor_copy(QT3[:, ts(t, SQT)], p[0:64, :])
                nc.vector.tensor_copy(KT3hi[64:128, ts(t, SQT)], p[64:128, :])
                nc.sync.dma_start(KT3[:, ts(t, SQT)],
                                  KT3hi[64:128, ts(t, SQT)])
                nc.sync.dma_start(QT3hi[64:128, ts(t, SQT)],
                                  QT3[:, ts(t, SQT)])

            def proj_v(t, tag="s", bufs=2):
                p = psum.tile([P, 192], fp32, tag=tag, name="vp", bufs=bufs)
                for k in range(NK):
                    nc.tensor.matmul(p[:], xTbs[t // 4][:, k, ts(t % 4, P)],
                                     wvb[:, k, :],
                                     start=(k == 0), stop=(k == NK - 1))
                nc.vector.tensor_copy(V_all[:, t, :, 0:64],
                                      p[:].rearrange("p (h e) -> p h e", h=3))

            # ---------------- upfront: only what scores(j=0,i=0) needs ----
            proj_k2(0)
            proj_q2(0)

            # ---------------- attention ----------------
            # pend_work holds closures popped (traced) inside attn(j)'s
            # loops.  Anything attn(j+1) READS must be traced before attn(j)
            # ends: proj(j+1) is seeded before attn(j) starts; out-proj(j)
            # and norm_solo(j) are appended at the end of attn(j) and
            # consumed during attn(j+1).
            pend_work = deque()

            def pop_work():
                if pend_work:
                    f = pend_work.popleft()
                    if f is not None:
                        f()

            def proj_items(t):
                # alternate the two 1-bank scratch rings ("o"/"zs") so two
                # evacuation chains run in parallel instead of serializing
                # on a single PSUM bank.
                return [lambda: proj_q2(t, "o", 1),
                        lambda: proj_k2(t, "zs", 1),
                        lambda: proj_qk3(t, "o", 1)] + \
                       [(lambda v, g: lambda: proj_v(v, g, 1))(
                           v, "zs" if v % 2 else "o")
                        for v in range(4 * t, 4 * t + 4)]

            # j=0 still needs V tiles 0:4 + qk3(0); fold them into the
            # closure stream so attention starts right after q2/k2(0).
            pend_work.extend([(lambda v, g: lambda: proj_v(v, g, 1))(
                v, "zs" if v % 2 else "o") for v in range(4)])
            pend_work.append(lambda: proj_qk3(0, "o", 1))
            pend_work.extend(proj_items(1))

            for j in range(NJ):
                n_i = 4 * j + 4
                n_m = n_i // 2
                sl = ts(j, SQT)
                zp0 = psum.tile([P, SQT], fp32, tag="z", name="zp0", bufs=2)
                zp1 = psum.tile([P, SQT], fp32, tag="z", name="zp1", bufs=2)
                zT2 = zcol.tile([P, SQT], bf16, tag="zT2", name="zT2")
                zT3 = zcol.tile([64, SQT], bf16, tag="zT3", name="zT3")

                def col0_of(i, j=j):
                    return P * (i - 4 * j) if i >= 4 * j else 0

                def s_pair_start(i, j=j):
                    col0 = col0_of(i)
                    s_ps = psum.tile([P, 2, SQT], fp32, tag="s", name="s_ps",
                                     bufs=2)
                    for h in range(2):
                        nc.tensor.matmul(
                            s_ps[:, h, col0:SQT],
                            KT2[64 * h:64 * h + 64, ts(i, P)],
                            QT2[64 * h:64 * h + 64,
                                ds(SQT * j + col0, SQT - col0)],
                            start=True, stop=True)
                    return s_ps, col0

                def s_solo_start(m, j=j):
                    # batch m covers sk tiles i0=2m (rows 0:64 of the PE,
                    # KT3/QT3) and i1=2m+1 (rows 64:128, KT3hi/QT3hi); the
                    # two matmuls run concurrently (distinct row groups).
                    i0, i1 = 2 * m, 2 * m + 1
                    c0, c1 = col0_of(i0), col0_of(i1)
                    s_ps = psum.tile([P, 2, SQT], fp32, tag="s", name="s_ps3",
                                     bufs=2)
                    nc.tensor.matmul(
                        s_ps[:, 0, c0:SQT],
                        KT3[:, ts(i0, P)],
                        QT3[:, ds(SQT * j + c0, SQT - c0)],
                        start=True, stop=True)
                    nc.tensor.matmul(
                        s_ps[:, 1, c1:SQT],
                        KT3hi[64:128, ts(i1, P)],
                        QT3hi[64:128, ds(SQT * j + c1, SQT - c1)],
                        start=True, stop=True)
                    return s_ps, c0, c1

                def exp_mask_pair(i, pend, j=j):
                    s_ps, col0 = pend
                    E_t = epool.tile([P, 2, SQT], bf16, tag="E", name="E_t")
                    nc.scalar.activation(E_t[:, :, col0:SQT],
                                         s_ps[:, :, col0:SQT], Exp, scale=0.125)
                    if i >= 4 * j:
                        nc.vector.tensor_tensor(
                            E_t[:, :, col0:col0 + P], E_t[:, :, col0:col0 + P],
                            maskb[:, None, :].to_broadcast((P, 2, P)), Mult)
                    return E_t, col0

                def exp_mask_solo(m, pend, j=j):
                    s_ps, c0, c1 = pend
                    i0, i1 = 2 * m, 2 * m + 1
                    E_t = epool.tile([P, 2, SQT], bf16, tag="E", name="E_t3")
                    nc.scalar.activation(E_t[:, :, c0:SQT],
                                         s_ps[:, :, c0:SQT], Exp, scale=0.125)
                    if i0 >= 4 * j:
                        nc.vector.tensor_tensor(
                            E_t[:, 0, c0:c0 + P], E_t[:, 0, c0:c0 + P],
                            maskb[:], Mult)
                    if i1 >= 4 * j:
                        nc.vector.tensor_tensor(
                            E_t[:, 1, c1:c1 + P], E_t[:, 1, c1:c1 + P],
                            maskb[:], Mult)
                    return E_t, c0, c1

                # ---- pair i-loop ----
                pend = s_pair_start(0)
                for i in range(n_i):
                    E_t, col0 = exp_mask_pair(i, pend)
                    if i + 1 < n_i:
                        pend = s_pair_start(i + 1)
                    pop_work()
                    for h, zph in ((0, zp0), (1, zp1)):
                        nc.tensor.matmul(
                            zph[0:65, col0:SQT],
                            V_all[:, i, h, :],
                            E_t[:, h, col0:SQT],
                            start=(i == 0), stop=(i == n_i - 1),
                            skip_group_check=True)
                    pop_work()

                # ---- normalize: 1/ones-row straight from PSUM, broadcast
                # across partitions via a K=1 outer-product matmul.
                def norm_stages(z_ps, dst_direct, shift_hi, zT2=zT2,
                                otag="o", obufs=1, via_pe=False):
                    # 4 separately-popped stages so no engine queues an op
                    # whose input is still being produced cross-engine.  The
                    # partition broadcast rides a DRAM round-trip DMA whose
                    # latency hides between pops; at the tail (nothing left
                    # to hide behind) it uses a K=1 PE outer product instead.
                    st = {}

                    def a():
                        st['drow'] = zsb.tile(
                            [1, SQT], bf16 if via_pe else fp32, tag="rb",
                            name="drow")
                        nc.vector.tensor_copy(st['drow'][:], z_ps[64:65, :])
                        if not via_pe:
                            st['dscr'] = dram.tile([1, SQT], fp32,
                                                   name="dscr")
                            nc.gpsimd.dma_start(st['dscr'][:], st['drow'][:])

                    def b():
                        if via_pe:
                            st['dps'] = psum.tile([64, SQT], fp32, tag=otag,
                                                  name="dps", bufs=obufs)
                            nc.tensor.matmul(st['dps'][:], onesb[:],
                                             st['drow'][:], start=True,
                                             stop=True)
                        else:
                            st['draw'] = zsb.tile([64, SQT], fp32, tag="rbw",
                                                  name="draw")
                            nc.gpsimd.dma_start(
                                st['draw'][:],
                                st['dscr'][:].to_broadcast((64, SQT)))

                    def c():
                        if via_pe:
                            st['draw'] = zsb.tile([64, SQT], fp32, tag="rbw",
                                                  name="draw")
                            nc.vector.tensor_copy(st['draw'][:], st['dps'][:])
                        st['rbb'] = zsb.tile([64, SQT], fp32, tag="rbb",
                                             name="rbb")
                        nc.vector.reciprocal_approx_fast(st['rbb'][:],
                                                         st['draw'][:])

                    def d():
                        if shift_hi:
                            z1t = zcol.tile([64, SQT], bf16, tag="z1t",
                                            name="z1t")
                            nc.vector.tensor_tensor(z1t[:], z_ps[0:64, :],
                                                    st['rbb'][:], Mult)
                            nc.sync.dma_start(zT2[64:128, :], z1t[:])
                        else:
                            nc.vector.tensor_tensor(dst_direct, z_ps[0:64, :],
                                                    st['rbb'][:], Mult)

                    return [a, b, c, d]

                n0 = norm_stages(zp0, zT2[0:64, :], False,
                                 via_pe=(j == NJ - 1))
                n1 = norm_stages(zp1, None, True, via_pe=(j == NJ - 1))
                local_work = deque(
                    [n0[0], n1[0], n0[1], n0[2], n1[1], n0[3], n1[2], n1[3]])

                # ---- solo i-loop (2 sk tiles per step) ----
                # zs shares its bank with the out-proj scratch ring: the
                # bank carries oproj(j-1) chains during the pair loop, then
                # zs(j) accumulation here (allocation order serializes it).
                zs = psum.tile([P, SQT], fp32, tag="zs", name="zs", bufs=1)
                pend = s_solo_start(0)
                for m in range(n_m):
                    i0, i1 = 2 * m, 2 * m + 1
                    E_t, c0, c1 = exp_mask_solo(m, pend)
                    if m + 1 < n_m:
                        pend = s_solo_start(m + 1)
                    if local_work:
                        local_work.popleft()()
                    else:
                        pop_work()
                    nc.tensor.matmul(
                        zs[0:65, c0:SQT],
                        V_all[:, i0, 2, :],
                        E_t[:, 0, c0:SQT],
                        start=(m == 0), stop=False,
                        skip_group_check=True)
                    if local_work:
                        local_work.popleft()()
                    else:
                        pop_work()
                    nc.tensor.matmul(
                        zs[0:65, c1:SQT],
                        V_all[:, i1, 2, :],
                        E_t[:, 1, c1:SQT],
                        start=False, stop=(m == n_m - 1),
                        skip_group_check=True)
                    pop_work()

                while local_work:
                    local_work.popleft()()

                # ---- deferred work for the next j ----
                tail = (j == NJ - 1)

                norm_solo_items = norm_stages(
                    zs, zT3[:], False,
                    otag=("s" if tail else "o"),
                    obufs=(2 if tail else 1), via_pe=tail)

                def oproj(c, j=j, zT2=zT2, zT3=zT3, tail=tail):
                    # tail out-projs ping-pong on the freed "s" banks;
                    # oproj(NJ-2) pops during the last solo loop (after
                    # zs(NJ-1) is allocated) so it takes the then-free "o"
                    # ring; otherwise the shared "zs" scratch ring keeps the
                    # "o" ring free for projection evacuations.
                    otag = ("s" if tail else ("o" if j >= NJ - 3 else "zs"))
                    obufs = 2 if tail else 1
                    row = ds(SQT * j + P * c, P)
                    stage = zsb.tile([P, D], bf16, tag="ost", name="ost",
                                     bufs=3)
                    if tail:
                        # one 2-bank tile per chunk: the next chunk's MMs
                        # overlap this chunk's evacuation copies.
                        ot = psum.tile([P, 2, SQT], fp32, tag=otag,
                                       name="otl", bufs=2)
                        o1, o2 = ot[:, 0, :], ot[:, 1, 0:256]
                    else:
                        o1 = psum.tile([P, SQT], fp32, tag=otag, name="o1",
                                       bufs=obufs)[:]
                        o2 = None
                    nc.tensor.matmul(o1, zT2[:, ts(c, P)], wo2b[:, 0:512],
                                     start=True, stop=False,
                                     skip_group_check=True)
                    nc.tensor.matmul(o1, zT3[:, ts(c, P)], wo3b[:, 0:512],
                                     start=False, stop=True,
                                     skip_group_check=True)
                    nc.vector.tensor_copy(stage[:, 0:512], o1)
                    if o2 is None:
                        o2 = psum.tile([P, 256], fp32, tag=otag, name="o2",
                                       bufs=obufs)[:]
                    nc.tensor.matmul(o2, zT2[:, ts(c, P)],
                                     wo2b[:, 512:768],
                                     start=True, stop=False,
                                     skip_group_check=True)
                    nc.tensor.matmul(o2, zT3[:, ts(c, P)],
                                     wo3b[:, 512:768],
                                     start=False, stop=True,
                                     skip_group_check=True)
                    nc.vector.tensor_copy(stage[:, 512:768], o2)
                    (nc.sync if tail else nc.gpsimd).dma_start(
                        out[row, :], stage[:])

                def mk(f, *a):
                    return lambda: f(*a)

                oq = [mk(oproj, 0), mk(oproj, 1), mk(oproj, 2), mk(oproj, 3)]
                if j == NJ - 3:
                    # j2's loops are already closure-saturated by proj(3):
                    # park half of oproj(1) past them, into pair j3's many
                    # empty slots (pops during j2 = 24 pair + ~10 solo).
                    items = list(proj_items(j + 2)) + oq[0:2] +                         [None] * 12 + oq[2:4]
                elif j < NJ - 2:
                    rest = iter(proj_items(j + 2))
                    items = [next(rest), next(rest)]
                    for idx, o_cl in enumerate(oq):
                        items.append(o_cl)
                        nxt = next(rest, None)
                        if nxt is not None:
                            items.append(nxt)
                    items += list(rest)
                elif j == NJ - 2:
                    items = [None] * 2 + oq
                else:
                    items = oq
                pend_work.extend(norm_solo_items)
                pend_work.extend(items)

            # ---- drain remaining deferred work (j=3 tail) ----
            while pend_work:
                f = pend_work.popleft()
                if f is not None:
                    f()

    nc.compile()
    return nc


def _get_program():
    global _PROGRAM
    if _PROGRAM is None:
        _PROGRAM = _build_program()
    return _PROGRAM


def kernel(x, W_Q, W_K, W_V, W_O, b_Q, b_K, b_V, b_O):
    global LAST_RESULTS
    _install_ntff_shim()
    from concourse import bass_utils

    x = np.asarray(x, dtype=np.float32)
    W_Q = np.asarray(W_Q, dtype=np.float32)
    W_K = np.asarray(W_K, dtype=np.float32)
    W_V = np.asarray(W_V, dtype=np.float32)
    W_O = np.asarray(W_O, dtype=np.float32)
    b_Q = np.asarray(b_Q, dtype=np.float32)
    b_K = np.asarray(b_K, dtype=np.float32)
    b_V = np.asarray(b_V, dtype=np.float32)
    b_O = np.asarray(b_O, dtype=np.float32)
    assert not (np.any(b_Q) or np.any(b_K) or np.any(b_V)), \
        "kernel assumes zero QKV biases (problem spec fill=zeros)"

    nc = _get_program()

    def bf(a):
        return np.ascontiguousarray(a.astype(BF16))

    def dev_w(w):
        # [768, e] -> [128, 6, e]: chunk k rows on axis 1, partitions on axis 0
        e = w.shape[1]
        return bf(w.reshape(NK, P, e).transpose(1, 0, 2))

    mask = bf(np.triu(np.ones((P, P), dtype=np.float32)))
    # x[b].T is [768, 2048]; stage as [NJ, 128, 6, 512] so each 512-column
    # chunk is contiguous per partition (fast DMA, chunk 0 lands first)
    xTs = [bf(x[b].T.reshape(NK, P, NJ, SQT).transpose(2, 1, 0, 3))
           for b in range(B)]

    in_maps = []
    for c in range(N_CORES):
        b, g = c // 4, c % 4
        hs = [3 * g, 3 * g + 1, 3 * g + 2]
        in_maps.append({
            "xT": xTs[b],
            "wq2": dev_w(np.concatenate([W_Q[hs[0]], W_Q[hs[1]]], axis=1)),
            "wk2": dev_w(np.concatenate([W_K[hs[0]], W_K[hs[1]]], axis=1)),
            "wqk3": dev_w(np.concatenate([W_Q[hs[2]], W_K[hs[2]]], axis=1)),
            "wv": dev_w(np.concatenate(
                [W_V[hs[0]], W_V[hs[1]], W_V[hs[2]]], axis=1)),
            "wo2": bf(np.concatenate([W_O[hs[0]], W_O[hs[1]]], axis=0)),
            "wo3": bf(W_O[hs[2]]),
            "mask": mask,
        })

    res = bass_utils.run_bass_kernel_spmd(
        nc, in_maps, core_ids=list(range(N_CORES)),
        trace=bool(os.environ.get("BASS_TRACE")))
    LAST_RESULTS = res

    parts = [res.results[c]["out"].astype(np.float32) for c in range(N_CORES)]
    full = np.stack([
        parts[0] + parts[1] + parts[2] + parts[3],
        parts[4] + parts[5] + parts[6] + parts[7],
    ], axis=0)
    if np.any(b_O):
        full = full + b_O
    return full.astype(np.float32)
